# revision 10
# baseline (speedup 1.0000x reference)
"""
Bass/Trainium2 kernel for nn_Attention_72456098284196.

Attention module: QKV projections + partial rotary (first 32 of 64 head
channels, all heads) + softmax attention.  B=2, T=2048, C=1024, H=16, D=64.

Sharding: 8 NeuronCores = 2 batches x 4 head-groups (4 heads each).
Pure tensor/batch parallel -> no collectives; host slices inputs and
concatenates outputs.

Host prep: x / W are cast to bf16 and transposed; the rotary cos/sin
tables are precomputed from the position inputs ([128, T] bf16, one row
per channel of a 2-head c-tile; the splice sign is folded into the sin
table so the device-side splice is a plain partition pair-swap).

Device-side dataflow (per core, matmuls bf16 with fp32 PSUM accum):
  DMA: few large 128-partition transfers; w/x on the SP+Act rings,
       rotary tables + bias columns on the Pool ring.
  qT[c,t] = WqT.T @ xqT   (c = 4 heads x 64 ch, two 128-partition c-tiles)
  rotary:  qb = bf16(q + b) evacuates PSUM once; shuffling qb gives
           swap(q)+swap(b) for free, so the chain is one PSUM-read op
           plus shuffle/mul/mul/add in fast all-SBUF bf16 mode:
           rq = qb*cos + shuffle(qb)*sin_signed   (~2us per 512-chunk)
  scores^T[s,t] = rkT.T @ rqT per head: one [k=64, m=128, n=512] matmul
           per (head, psum bank) in 64x128 row-tiled PE mode
           (tile_position=(h*64,0)) -- matmul engine time is n cycles
           regardless of k/m, so m=128 halves score matmul time vs the
           64-wide quadrant packing, and the 64-row ldweights stay
           hidden under the 512-cycle streams.
  expT = Exp(scores^T / 8) on ScalarE -> bf16.  ScalarE is the pacing
           engine (~1.1us per [128,1024] tile, 128 tiles = ~142us); the
           schedule keeps it fed: window it0 weaves the k01/q01
           projections and all 16 v-projections into the PE idle gaps
           between score tiles, pair-1 projections ride psva slots
           right after, and windows it1..3 interleave AV(it-1).
  outT[d,t] accum over s of [v | 1].T @ expT  (M=65: row 64 accumulates
           the softmax denominator for free)
  normalize: recip(denominator row) via fast approx, broadcast 1->64
           partitions (GpSimd mid-stream; PE outer-product on the tail
           where the scores PSUM banks are free), multiply, DMA out.
"""

import math
import sys

import numpy as np

if "/opt/trn_rl_repo" not in sys.path:
    sys.path.insert(0, "/opt/trn_rl_repo")

import concourse.bass as bass  # noqa: E402
import concourse.mybir as mybir  # noqa: E402
import concourse.tile as tile  # noqa: E402
from concourse import bacc  # noqa: E402
from concourse.bass_utils import run_bass_kernel_spmd  # noqa: E402

B, T, C = 2, 2048, 1024
NUM_HEADS = 16
HEAD_DIM = 64
N_CORES = 8
HEADS_PER_CORE = NUM_HEADS // (N_CORES // B)  # 4
CO = HEADS_PER_CORE * HEAD_DIM  # 256 out channels per core
N_ROT = 32  # rotated channels per head
MAX_WAVELENGTH = 8192.0

F32 = mybir.dt.float32
BF16 = mybir.dt.bfloat16
NPBF16 = mybir.dt.np(BF16)

P = 128  # partitions
TCH = 512  # matmul N chunk (1 PSUM bank)
KCH = C // P  # 8 contraction chunks
NCT = CO // P  # 2 c-tiles (each = 2 heads x 64)
NST = T // P  # 16 s tiles
SCALE = 1.0 / math.sqrt(HEAD_DIM)
TH = 1024  # attention t-half width / x DMA half width
XH = 2


def _inv_freq() -> np.ndarray:
    """[32] inverse frequencies (pairs repeated), matching the reference."""
    num_bands = N_ROT // 2  # 16
    freq = MAX_WAVELENGTH ** (
        2.0 / N_ROT * np.linspace(0.0, num_bands, num_bands, dtype=np.float64)
    )
    return np.repeat(1.0 / freq, 2)  # [32]


def _rot_tables(pos: np.ndarray) -> tuple[np.ndarray, np.ndarray]:
    """cos/sin tables [128, T] bf16 for a 2-head c-tile.

    Rows r in [0,32) and [64,96): rotary channels (cos/sin of pos*invf);
    other rows: cos=1, sin=0 (passthrough).  The splice negation is
    folded into sin: even channels get -sin so that
    rq = q*cos + pairswap(q)*sin_signed.
    """
    inv = _inv_freq()  # [32]
    rad = pos.astype(np.float64)[None, :] * inv[:, None]  # [32, T]
    cos32 = np.cos(rad)
    sin32 = np.sin(rad)
    sign = np.where(np.arange(N_ROT) % 2 == 0, -1.0, 1.0)[:, None]
    sin32 = sin32 * sign
    cos = np.zeros((P, pos.shape[0]), np.float64)
    sin = np.zeros((P, pos.shape[0]), np.float64)
    for o in (0, 64):
        cos[o : o + N_ROT] = cos32
        sin[o : o + N_ROT] = sin32
        cos[o + N_ROT : o + 64] = 1.0
    return cos.astype(NPBF16), sin.astype(NPBF16)


_SWAP_MASK = [i ^ 1 for i in range(32)]  # pair swap within each 32-quadrant


def build_bass() -> bass.Bass:
    nc = bacc.Bacc()

    # x / w are host-prepacked partition-major ([P, KCH, ...]) so each
    # DMA moves 128 x 16KB contiguous per-partition runs.
    xq_ext = [
        nc.declare_dram_parameter(f"xqT{h}", [P, KCH, TH], BF16, isOutput=False)
        for h in range(XH)
    ]
    xkv_ext = [
        nc.declare_dram_parameter(f"xkvT{h}", [P, KCH, TH], BF16, isOutput=False)
        for h in range(XH)
    ]
    wq_ext = nc.declare_dram_parameter("wqT", [P, KCH, CO], BF16, isOutput=False)
    wk_ext = nc.declare_dram_parameter("wkT", [P, KCH, CO], BF16, isOutput=False)
    wv_ext = nc.declare_dram_parameter("wvT", [P, KCH, CO], BF16, isOutput=False)
    bias_ext = {}
    for nm in ("bq", "bk", "bv"):
        bias_ext[nm] = nc.declare_dram_parameter(nm, [CO, 1], F32, isOutput=False)
    tab_ext = {}
    for nm in ("cosq", "sinq", "cosk", "sink"):
        tab_ext[nm] = nc.declare_dram_parameter(nm, [P, T], BF16, isOutput=False)
    out_ext = nc.declare_dram_parameter("out", [CO, T], F32, isOutput=True)

    ExpF = mybir.ActivationFunctionType.Exp

    with tile.TileContext(nc) as tc:
        from contextlib import ExitStack

        stack_all = ExitStack()
        consts = stack_all.enter_context(tc.tile_pool(name="consts", bufs=1))
        persist = stack_all.enter_context(tc.tile_pool(name="persist", bufs=1))
        xw = stack_all.enter_context(tc.tile_pool(name="xw", bufs=1))
        projtmp = stack_all.enter_context(tc.tile_pool(name="projtmp", bufs=2))
        scp = stack_all.enter_context(
            tc.tile_pool(name="scp", bufs=2, space="PSUM")
        )
        expp = stack_all.enter_context(tc.tile_pool(name="expp", bufs=31))
        outp = stack_all.enter_context(tc.tile_pool(name="outp", bufs=2))
        smallp = stack_all.enter_context(tc.tile_pool(name="small", bufs=1))

        # ---------------- PE warmup + exp table preload ----------------
        # ~40 garbage matmuls keep the PE HAM clock warm through the DMA
        # wait so the first real projections run at 2.4GHz; a tiny Exp
        # activation pulls the ACT_TABLE_LOAD off the first-score path.
        warm = consts.tile([P, TCH], BF16, tag="warm")
        nc.vector.memset(warm[:], 1.0)
        ones64 = consts.tile([1, HEAD_DIM], BF16, tag="ones64")
        nc.vector.memset(ones64[:], 1.0)
        wtiny = consts.tile([1, 2], F32, tag="wtiny")
        nc.vector.memset(wtiny[:], 0.0)
        nc.scalar.activation(wtiny[:], wtiny[:],
                             mybir.ActivationFunctionType.Exp)

        # ---------------- input DMAs: few, large, 128-partition ----------
        # SP ring: xq h0, xkv h0, xq h1, xkv h1 halves (1MB each).
        # Act ring: wq, the other x halves, wk, wv.
        # Pool ring: bias columns first, then the cos/sin tables, then bv.
        x_sb = {}
        for name in ("q", "kv"):
            for h in range(XH):
                x_sb[(name, h)] = xw.tile([P, KCH, TH], BF16, tag=f"x{name}{h}",
                                          name=f"x{name}{h}")

        def load_x(name, exts, h):
            # split each 2MB half across both HWDGE rings (1MB each)
            half_elems = (KCH // 2) * TH
            nc.sync.dma_start(
                out=x_sb[(name, h)][:, 0 : KCH // 2, :],
                in_=bass.AP(tensor=exts[h], offset=0,
                            ap=[[KCH * TH, P], [1, half_elems]]),
            )
            nc.scalar.dma_start(
                out=x_sb[(name, h)][:, KCH // 2 : KCH, :],
                in_=bass.AP(tensor=exts[h], offset=half_elems,
                            ap=[[KCH * TH, P], [1, half_elems]]),
            )

        w_sb = {}
        tabs = {}

        def load_w(name, ext):
            wb = xw.tile([P, KCH, CO], BF16, tag=f"w{name}", name=f"w{name}")
            nc.scalar.dma_start(
                out=wb[:],
                in_=bass.AP(tensor=ext, offset=0,
                            ap=[[KCH * CO, P], [1, KCH * CO]]),
            )
            w_sb["w" + name] = wb

        def load_tab(nm):
            t_ = consts.tile([P, T], BF16, tag=nm)
            nc.gpsimd.dma_start(out=t_[:], in_=tab_ext[nm][:, :])
            tabs[nm] = t_

        # tiny bias columns first on the Pool ring so rotary never waits
        bias_cols = {}
        for nm in ("bq", "bk"):
            for ct in range(NCT):
                t_ = consts.tile([P, 1], F32, tag=f"{nm}{ct}")
                nc.gpsimd.dma_start(
                    out=t_[:], in_=bias_ext[nm][ct * P : (ct + 1) * P, :]
                )
                bias_cols[(nm, ct)] = t_

        # startup-critical order: weights, xq h0, xkv h0 on the fast
        # rings; tables stream on the Pool ring in parallel.
        load_w("q", wq_ext)
        load_x("q", xq_ext, 0)
        load_w("k", wk_ext)
        load_x("kv", xkv_ext, 0)
        load_tab("cosq")
        load_tab("sinq")
        load_tab("cosk")
        load_tab("sink")
        load_w("v", wv_ext)
        load_x("q", xq_ext, 1)
        load_x("kv", xkv_ext, 1)

        bvb_sb = consts.tile([P, CO], F32, tag="bvb")
        nc.gpsimd.dma_start(
            out=bvb_sb[:],
            in_=bass.AP(tensor=bias_ext["bv"], offset=0, ap=[[0, P], [1, CO]]),
        )
        # persistent rotated q/k and v tiles
        rot_sb = {}
        for name in ("q", "k"):
            for ct in range(NCT):
                for hf in range(XH):
                    rot_sb[(name, ct, hf)] = persist.tile(
                        [P, TH], BF16, tag=f"r{name}{ct}{hf}",
                        name=f"r{name}{ct}{hf}"
                    )
        v_sb = [
            persist.tile([P, HEADS_PER_CORE, HEAD_DIM + 1], BF16,
                         tag=f"v{st}", name=f"v{st}")
            for st in range(NST)
        ]

        stack_p = ExitStack()
        projp = stack_p.enter_context(
            tc.tile_pool(name="projp", bufs=4, space="PSUM")
        )

        # ------------- q/k projection + rotary (one t-half) -------------
        def proj_alloc(pool, tag, nm):
            return [
                pool.tile([P, TCH], F32, tag=tag, name=f"pj{nm}_{i}")
                for i in range(2)
            ]

        def proj_mms(name, xsrc, ct, half, pss, ks):
            for k in ks:
                for i in range(2):
                    nc.tensor.matmul(
                        pss[i][:],
                        w_sb["w" + name][:, k, ct * P : (ct + 1) * P],
                        x_sb[(xsrc, half)][:, k, i * TCH : (i + 1) * TCH],
                        start=(k == 0),
                        stop=(k == KCH - 1),
                    )

        def proj_rot(name, ct, half, pss):
            # qb = bf16(q + b) evacuates PSUM; shuffle(qb) = swap(q)+swap(b)
            # so the sin path needs no separate swapped-bias table.
            dst = rot_sb[(name, ct, half)]
            cos_t = tabs["cos" + name]
            sin_t = tabs["sin" + name]
            for i in range(2):
                ps = pss[i]
                tsl = slice(half * TH + i * TCH, half * TH + (i + 1) * TCH)
                dsl = slice(i * TCH, (i + 1) * TCH)
                qb = projtmp.tile([P, TCH], BF16, tag="qb",
                                  name=f"qb{name}{ct}{half}{i}")
                nc.vector.tensor_scalar_add(
                    qb[:], ps[:], bias_cols[("b" + name, ct)][:]
                )
                qsw = projtmp.tile([P, TCH], BF16, tag="qsw",
                                   name=f"qsw{name}{ct}{half}{i}")
                nc.vector.stream_shuffle(qsw[:], qb[:], _SWAP_MASK)
                t2 = projtmp.tile([P, TCH], BF16, tag="rot2",
                                  name=f"t2{name}{ct}{half}{i}")
                nc.vector.tensor_mul(t2[:], qsw[:], sin_t[:, tsl])
                nc.vector.tensor_mul(dst[:, dsl], qb[:], cos_t[:, tsl])
                nc.vector.tensor_add(dst[:, dsl], dst[:, dsl], t2[:])

        def proj_group(name, xsrc, ct, half, pool=None, tag="pj"):
            pss = proj_alloc(pool or projp, tag, f"{name}{ct}{half}")
            proj_mms(name, xsrc, ct, half, pss, range(KCH))
            proj_rot(name, ct, half, pss)
            return pss

        wps = projp.tile([P, TCH], F32, tag="pj", name="warmps")
        for i in range(40):
            nc.tensor.matmul(
                wps[:], warm[:, 0:P], warm[:],
                start=True, stop=True,
            )

        # v projection ([128,256] fits a projp slot)
        def emit_v_proj(sts):
            for st in sts:
                vt = v_sb[st]
                psv = projp.tile([P, CO], F32, tag="pj", name=f"psv{st}")
                half, col = divmod(st * P, TH)
                for k in range(KCH):
                    nc.tensor.matmul(
                        psv[:],
                        x_sb[("kv", half)][:, k, col : col + P],
                        w_sb["wv"][:, k, :],
                        start=(k == 0),
                        stop=(k == KCH - 1),
                    )
                nc.vector.tensor_add(
                    vt[:, :, 0:HEAD_DIM],
                    psv[:].rearrange("p (h d) -> p h d", h=HEADS_PER_CORE),
                    bvb_sb[:].rearrange("p (h d) -> p h d", h=HEADS_PER_CORE),
                )
                nc.vector.memset(vt[:, :, HEAD_DIM : HEAD_DIM + 1], 1.0)

        # ---------------- attention ----------------
        ITERS = [(p_, t_) for p_ in range(NCT) for t_ in range(2)]

        def scores_exp(it, st):
            pair, th = ITERS[it]
            rk = rot_sb[("k", pair, st // 8)]
            rq = rot_sb[("q", pair, th)]
            so = (st % 8) * P
            pss = [
                scp.tile([P, TH], F32, tag="sc", name=f"sc{it}_{st}_{h}")
                for h in range(2)
            ]
            # one [k=64, m=128, n=512] matmul per (head, bank): 64x128
            # row-tiled mode; h0 on row-tile 0, h1 on row-tile 64.
            for tcc in range(2):
                psl = slice(tcc * TCH, (tcc + 1) * TCH)
                for h in range(2):
                    nc.tensor.matmul(
                        pss[h][:, psl],
                        rk[h * 64 : (h + 1) * 64, so : so + P],
                        rq[h * 64 : (h + 1) * 64, psl],
                        start=True, stop=True,
                        tile_position=(h * 64, 0),
                    )
            etiles = []
            for h in range(2):
                e = expp.tile([P, TH], BF16, tag="exp", name=f"e{it}_{st}_{h}")
                nc.scalar.activation(e[:], pss[h][:], ExpF, scale=SCALE)
                etiles.append(e)
            return etiles

        def av_mms(it, st, vps, etiles):
            pair, th = ITERS[it]
            for sub in range(2):
                h = pair * 2 + sub
                e = etiles[st][sub]
                for tcc in range(2):
                    psl = slice(tcc * TCH, (tcc + 1) * TCH)
                    nc.tensor.matmul(
                        vps[sub][:, psl],
                        v_sb[st][:, h, :],
                        e[:, psl],
                        start=(st == 0),
                        stop=(st == NST - 1),
                    )

        def epilogue_sub(it, vps, sub, tail=False):
            pair, th = ITERS[it]
            h = pair * 2 + sub
            vcp = outp.tile([HEAD_DIM + 1, TH], F32, tag="vcp",
                            name=f"vcp{it}_{sub}")
            nc.vector.tensor_copy(vcp[:], vps[sub][:])
            dn = smallp.tile([1, TH], F32, tag="dn",
                             name=f"dn{it}_{sub}")
            nc.sync.dma_start(
                out=dn[:], in_=vcp[HEAD_DIM : HEAD_DIM + 1, :]
            )
            if tail:
                # tail epilogue: the scores PSUM banks are free, so the
                # 1->64 broadcast is a PE outer product (ones64^T @ recb)
                # instead of two ~1us GpSimd broadcasts.
                nc.vector.reciprocal_approx_fast(out=dn[:], in_=dn[:])
                recb = smallp.tile([1, TH], BF16, tag="recb",
                                   name=f"recb{it}_{sub}")
                nc.vector.tensor_copy(recb[:], dn[:])
                bc = scp.tile([HEAD_DIM, TH], F32, tag="sc",
                              name=f"bc{it}_{sub}")
                for j in range(2):
                    nc.tensor.matmul(
                        bc[:, j * TCH : (j + 1) * TCH],
                        ones64[:],
                        recb[:, j * TCH : (j + 1) * TCH],
                        start=True, stop=True,
                    )
                nc.vector.tensor_mul(
                    vcp[0:HEAD_DIM, :], vcp[0:HEAD_DIM, :], bc[:]
                )
            else:
                nc.vector.reciprocal_approx_fast(out=dn[:], in_=dn[:])
                recb = smallp.tile([1, TH], BF16, tag="recb",
                                   name=f"recb{it}_{sub}")
                nc.vector.tensor_copy(recb[:], dn[:])
                # broadcast 1->64 partitions on GpSimd (keeps PE queue free)
                rcb = smallp.tile([HEAD_DIM, TH], BF16, tag="rcb",
                                  name=f"rcb{it}_{sub}")
                for j in range(2):
                    nc.gpsimd.partition_broadcast(
                        rcb[:, j * TCH : (j + 1) * TCH],
                        recb[:, j * TCH : (j + 1) * TCH],
                        channels=HEAD_DIM,
                    )
                nc.vector.tensor_mul(
                    vcp[0:HEAD_DIM, :], vcp[0:HEAD_DIM, :], rcb[:]
                )
            nc.sync.dma_start(
                out=out_ext[h * HEAD_DIM : (h + 1) * HEAD_DIM,
                            th * TH : (th + 1) * TH],
                in_=vcp[0:HEAD_DIM, :],
            )

        def epilogue(it, vps):
            for sub in range(2):
                epilogue_sub(it, vps, sub)

        # ---------------- window it0 with woven projections -------------
        # pair-0 th0 groups first (critical path to the first exp), then
        # the k01/q01 groups and all 16 v-projections are woven into the
        # ACT-paced idle gaps between score tiles so the PE queue never
        # stalls the exp pipeline and the DVE reaches every rotary early.
        proj_group("q", "q", 0, 0)
        proj_group("k", "kv", 0, 0)

        et = {0: []}
        et[0].append(scores_exp(0, 0))
        pss_k01 = proj_alloc(projp, "pj", "k01")
        proj_mms("k", "kv", 0, 1, pss_k01, range(0, 4))
        et[0].append(scores_exp(0, 1))
        proj_mms("k", "kv", 0, 1, pss_k01, range(4, KCH))
        proj_rot("k", 0, 1, pss_k01)
        et[0].append(scores_exp(0, 2))
        pss_q01 = proj_alloc(projp, "pj", "q01")
        proj_mms("q", "q", 0, 1, pss_q01, range(0, 4))
        et[0].append(scores_exp(0, 3))
        proj_mms("q", "q", 0, 1, pss_q01, range(4, KCH))
        proj_rot("q", 0, 1, pss_q01)
        vq = list(range(NST))
        for st in range(4, NST):
            et[0].append(scores_exp(0, st))
            n = 1 if st < 14 else (2 if st == 14 else 4)
            for _ in range(n):
                if vq:
                    emit_v_proj([vq.pop(0)])
        stack_p.close()

        psva = stack_all.enter_context(
            tc.tile_pool(name="psva", bufs=2, space="PSUM")
        )
        # pair-1 projections ride psva slots (so AV(0) is gated only on
        # these four groups, not on the whole projection pool)
        proj_group("q", "q", 1, 0, psva, "va")
        proj_group("k", "kv", 1, 0, psva, "va")
        proj_group("q", "q", 1, 1, psva, "va")
        proj_group("k", "kv", 1, 1, psva, "va")

        def new_vps(it):
            return [
                psva.tile([HEAD_DIM + 1, TH], F32, tag="va",
                          name=f"vacc{it}_{s}")
                for s in range(2)
            ]

        # windows it1..it3: each window runs the second half of AV(it-1),
        # its epilogue at mid-window (so the next vps allocates early),
        # then the first half of AV(it) chasing this window's exp.
        vps = {}
        vps[0] = new_vps(0)
        et[1] = []
        for st in range(8):
            et[1].append(scores_exp(1, st))
            av_mms(0, 2 * st, vps[0], et[0])
            av_mms(0, 2 * st + 1, vps[0], et[0])
        epilogue(0, vps[0])
        vps[1] = new_vps(1)
        for st in range(8, NST):
            et[1].append(scores_exp(1, st))
            av_mms(1, st - 8, vps[1], et[1])
        for it in (2, 3):
            et[it] = []
            for st in range(8):
                et[it].append(scores_exp(it, st))
                av_mms(it - 1, st + 8, vps[it - 1], et[it - 1])
            epilogue(it - 1, vps[it - 1])
            vps[it] = new_vps(it)
            for st in range(8, NST):
                et[it].append(scores_exp(it, st))
                av_mms(it, st - 8, vps[it], et[it])
        # trailing: second half of AV(3) for both subs first, then the
        # two epilogue chains (PE-broadcast variant) with nothing queued
        # behind them on the PE.
        pair3, th3 = ITERS[3]
        for sub in range(2):
            h3 = pair3 * 2 + sub
            for st in range(8, NST):
                for tcc in range(2):
                    psl = slice(tcc * TCH, (tcc + 1) * TCH)
                    nc.tensor.matmul(
                        vps[3][sub][:, psl],
                        v_sb[st][:, h3, :],
                        et[3][st][sub][:, psl],
                        start=False,
                        stop=(st == NST - 1),
                    )
        for sub in range(2):
            epilogue_sub(3, vps[3], sub, tail=True)

        stack_all.close()
    nc.finalize()
    return nc


def make_in_maps(x_q, x_kv, q_positions, kv_positions, Wq, bq, Wk, bk, Wv, bv):
    x_q = np.asarray(x_q, np.float32)
    x_kv = np.asarray(x_kv, np.float32)
    q_positions = np.asarray(q_positions, np.int32)
    kv_positions = np.asarray(kv_positions, np.int32)
    Wq, Wk, Wv = (np.asarray(w, np.float32) for w in (Wq, Wk, Wv))
    bq, bk, bv = (np.asarray(b, np.float32) for b in (bq, bk, bv))

    xqT = [np.ascontiguousarray(x_q[b_].T).astype(NPBF16) for b_ in range(B)]
    xkvT = [np.ascontiguousarray(x_kv[b_].T).astype(NPBF16) for b_ in range(B)]
    tabs = []
    for b_ in range(B):
        cq, sq = _rot_tables(q_positions[b_])
        ck, sk = _rot_tables(kv_positions[b_])
        tabs.append((cq, sq, ck, sk))

    in_maps = []
    for core in range(N_CORES):
        b_, hg = divmod(core, N_CORES // B)
        hsl = slice(hg * CO, (hg + 1) * CO)
        cq, sq, ck, sk = tabs[b_]
        def prepack(wT):  # [C, n] -> [P, KCH, n] partition-major
            n = wT.shape[1]
            return np.ascontiguousarray(
                wT.reshape(KCH, P, n).transpose(1, 0, 2)
            )

        m = {
            "wqT": prepack(Wq[hsl].T.astype(NPBF16)),
            "wkT": prepack(Wk[hsl].T.astype(NPBF16)),
            "wvT": prepack(Wv[hsl].T.astype(NPBF16)),
            "bq": np.ascontiguousarray(bq[hsl][:, None]),
            "bk": np.ascontiguousarray(bk[hsl][:, None]),
            "bv": np.ascontiguousarray(bv[hsl][:, None]),
            "cosq": cq, "sinq": sq, "cosk": ck, "sink": sk,
        }
        for h in range(XH):
            m[f"xqT{h}"] = prepack(xqT[b_][:, h * TH : (h + 1) * TH])
            m[f"xkvT{h}"] = prepack(xkvT[b_][:, h * TH : (h + 1) * TH])
        in_maps.append(m)
    return in_maps


_CACHED = {}


def kernel(x_q, x_kv, q_positions, kv_positions, Wq, bq, Wk, bk, Wv, bv):
    in_maps = make_in_maps(
        x_q, x_kv, q_positions, kv_positions, Wq, bq, Wk, bk, Wv, bv
    )
    if "nc" not in _CACHED:
        _CACHED["nc"] = build_bass()
    nc = _CACHED["nc"]

    res = run_bass_kernel_spmd(nc, in_maps, core_ids=list(range(N_CORES)))
    out = np.empty((B, T, C), np.float32)
    for core in range(N_CORES):
        b_, hg = divmod(core, N_CORES // B)
        out[b_, :, hg * CO : (hg + 1) * CO] = res.results[core]["out"].T
    return out


# revision 18
# speedup vs baseline: 1.0493x; 1.0493x over previous
"""
Bass/Trainium2 kernel for nn_Attention_72456098284196.

Attention module: QKV projections + partial rotary (first 32 of 64 head
channels, all heads) + softmax attention.  B=2, T=2048, C=1024, H=16, D=64.

Sharding: 8 NeuronCores = 2 batches x 4 head-groups (4 heads each).
Pure tensor/batch parallel -> no collectives; host slices inputs and
concatenates outputs.

Host prep: x / W are cast to bf16 and transposed; the rotary cos/sin
tables are precomputed from the position inputs ([128, T] bf16, one row
per channel of a 2-head c-tile; the splice sign is folded into the sin
table so the device-side splice is a plain partition pair-swap).

Device-side dataflow (per core, matmuls bf16 with fp32 PSUM accum):
  DMA: few large 128-partition transfers; w/x on the SP+Act rings,
       rotary tables + bias columns on the Pool ring.
  qT[c,t] = WqT.T @ xqT   (c = 4 heads x 64 ch, two 128-partition c-tiles)
  rotary:  qb = bf16(q + b) evacuates PSUM once; shuffling qb gives
           swap(q)+swap(b) for free, so the chain is one PSUM-read op
           plus shuffle/mul/mul/add in fast all-SBUF bf16 mode:
           rq = qb*cos + shuffle(qb)*sin_signed   (~2us per 512-chunk)
  scores^T[s,t] = rkT.T @ rqT per head: one [k=64, m=128, n=512] matmul
           per (head, psum bank) in 64x128 row-tiled PE mode
           (tile_position=(h*64,0)) -- matmul engine time is n cycles
           regardless of k/m, so m=128 halves score matmul time vs the
           64-wide quadrant packing, and the 64-row ldweights stay
           hidden under the 512-cycle streams.
  expT = Exp(scores^T / 8) on ScalarE -> bf16.  ScalarE is the pacing
           engine (~1.1us per [128,1024] tile, 128 tiles = ~142us); the
           schedule keeps it fed: window it0 weaves ALL remaining
           projections (v x16, k01/q01, pair-1) two-chunks-per-gap into
           the PE idle between score tiles, ordered by DMA arrival;
           AV(it) then runs one window after its exps, woven into the
           late gaps of window it+1.
  outT[d,t] accum over s of [v | 1].T @ expT  (M=65: row 64 accumulates
           the softmax denominator for free)
  normalize: recip(denominator row) via fast approx, broadcast 1->64
           partitions (GpSimd mid-stream; PE outer-product on the tail
           where the scores PSUM banks are free), multiply, DMA out.
"""

import math
import sys

import numpy as np

if "/opt/trn_rl_repo" not in sys.path:
    sys.path.insert(0, "/opt/trn_rl_repo")

import concourse.bass as bass  # noqa: E402
import concourse.mybir as mybir  # noqa: E402
import concourse.tile as tile  # noqa: E402
from concourse import bacc  # noqa: E402
from concourse.bass_utils import run_bass_kernel_spmd  # noqa: E402

B, T, C = 2, 2048, 1024
NUM_HEADS = 16
HEAD_DIM = 64
N_CORES = 8
HEADS_PER_CORE = NUM_HEADS // (N_CORES // B)  # 4
CO = HEADS_PER_CORE * HEAD_DIM  # 256 out channels per core
N_ROT = 32  # rotated channels per head
MAX_WAVELENGTH = 8192.0

F32 = mybir.dt.float32
BF16 = mybir.dt.bfloat16
NPBF16 = mybir.dt.np(BF16)

P = 128  # partitions
TCH = 512  # matmul N chunk (1 PSUM bank)
KCH = C // P  # 8 contraction chunks
NCT = CO // P  # 2 c-tiles (each = 2 heads x 64)
NST = T // P  # 16 s tiles
SCALE = 1.0 / math.sqrt(HEAD_DIM)
TH = 1024  # attention t-half width / x DMA half width
XH = 2


def _inv_freq() -> np.ndarray:
    """[32] inverse frequencies (pairs repeated), matching the reference."""
    num_bands = N_ROT // 2  # 16
    freq = MAX_WAVELENGTH ** (
        2.0 / N_ROT * np.linspace(0.0, num_bands, num_bands, dtype=np.float64)
    )
    return np.repeat(1.0 / freq, 2)  # [32]


def _rot_tables(pos: np.ndarray) -> tuple[np.ndarray, np.ndarray]:
    """cos/sin tables [128, T] bf16 for a 2-head c-tile.

    Rows r in [0,32) and [64,96): rotary channels (cos/sin of pos*invf);
    other rows: cos=1, sin=0 (passthrough).  The splice negation is
    folded into sin: even channels get -sin so that
    rq = q*cos + pairswap(q)*sin_signed.
    """
    inv = _inv_freq()  # [32]
    rad = pos.astype(np.float64)[None, :] * inv[:, None]  # [32, T]
    cos32 = np.cos(rad)
    sin32 = np.sin(rad)
    sign = np.where(np.arange(N_ROT) % 2 == 0, -1.0, 1.0)[:, None]
    sin32 = sin32 * sign
    cos = np.zeros((P, pos.shape[0]), np.float64)
    sin = np.zeros((P, pos.shape[0]), np.float64)
    for o in (0, 64):
        cos[o : o + N_ROT] = cos32
        sin[o : o + N_ROT] = sin32
        cos[o + N_ROT : o + 64] = 1.0
    return cos.astype(NPBF16), sin.astype(NPBF16)


_SWAP_MASK = [i ^ 1 for i in range(32)]  # pair swap within each 32-quadrant


def build_bass() -> bass.Bass:
    nc = bacc.Bacc()

    # x / w are host-prepacked partition-major ([P, KCH, ...]) so each
    # DMA moves 128 x 16KB contiguous per-partition runs.
    xq_ext = [
        nc.declare_dram_parameter(f"xqT{h}", [P, KCH, TH], BF16, isOutput=False)
        for h in range(XH)
    ]
    xkv_ext = [
        nc.declare_dram_parameter(f"xkvT{h}", [P, KCH, TH], BF16, isOutput=False)
        for h in range(XH)
    ]
    wq_ext = nc.declare_dram_parameter("wqT", [P, KCH, CO], BF16, isOutput=False)
    wk_ext = nc.declare_dram_parameter("wkT", [P, KCH, CO], BF16, isOutput=False)
    wv_ext = nc.declare_dram_parameter("wvT", [P, KCH, CO], BF16, isOutput=False)
    bias_ext = {}
    for nm in ("bq", "bk", "bv"):
        bias_ext[nm] = nc.declare_dram_parameter(nm, [CO, 1], F32, isOutput=False)
    tab_ext = {}
    for nm in ("cosq", "sinq", "cosk", "sink"):
        tab_ext[nm] = nc.declare_dram_parameter(nm, [P, T], BF16, isOutput=False)
    out_ext = nc.declare_dram_parameter("out", [CO, T], F32, isOutput=True)

    ExpF = mybir.ActivationFunctionType.Exp

    with tile.TileContext(nc) as tc:
        from contextlib import ExitStack

        stack_all = ExitStack()
        consts = stack_all.enter_context(tc.tile_pool(name="consts", bufs=1))
        persist = stack_all.enter_context(tc.tile_pool(name="persist", bufs=1))
        xw = stack_all.enter_context(tc.tile_pool(name="xw", bufs=1))
        projtmp = stack_all.enter_context(tc.tile_pool(name="projtmp", bufs=1))
        scp = stack_all.enter_context(
            tc.tile_pool(name="scp", bufs=2, space="PSUM")
        )
        expp = stack_all.enter_context(tc.tile_pool(name="expp", bufs=33))
        outp = stack_all.enter_context(tc.tile_pool(name="outp", bufs=2))
        smallp = stack_all.enter_context(tc.tile_pool(name="small", bufs=1))

        # ---------------- PE warmup + exp table preload ----------------
        # ~40 garbage matmuls keep the PE HAM clock warm through the DMA
        # wait so the first real projections run at 2.4GHz; a tiny Exp
        # activation pulls the ACT_TABLE_LOAD off the first-score path.
        warm = consts.tile([P, TCH], BF16, tag="warm")
        nc.vector.memset(warm[:], 1.0)
        ones64 = consts.tile([1, HEAD_DIM], BF16, tag="ones64")
        nc.vector.memset(ones64[:], 1.0)
        wtiny = consts.tile([1, 2], F32, tag="wtiny")
        nc.vector.memset(wtiny[:], 0.0)
        nc.scalar.activation(wtiny[:], wtiny[:],
                             mybir.ActivationFunctionType.Exp)

        # ------------- input DMAs spread over the three rings ------------
        # The Act-ring sequencer executes its DMA-issue instructions
        # in-order ahead of the exps, and each issue waits a prior
        # transfer's completion (ring credit) -- so the Act ring gets only
        # early-completing w/h0 pieces and is free for the exps from
        # ~20us on.  The SP ring carries the h0 head/tail plus all of h1;
        # the Pool ring carries biases, the four tables, and a late xq1
        # half.
        x_sb = {}
        for name in ("q", "kv"):
            for h in range(XH):
                x_sb[(name, h)] = xw.tile([P, KCH, TH], BF16, tag=f"x{name}{h}",
                                          name=f"x{name}{h}")

        def load_x_part(name, exts, h, k0, k1, eng):
            elems = (k1 - k0) * TH
            eng.dma_start(
                out=x_sb[(name, h)][:, k0:k1, :],
                in_=bass.AP(tensor=exts[h], offset=k0 * TH,
                            ap=[[KCH * TH, P], [1, elems]]),
            )

        w_sb = {}
        tabs = {}

        def load_w(name, ext, eng):
            wb = xw.tile([P, KCH, CO], BF16, tag=f"w{name}", name=f"w{name}")
            eng.dma_start(
                out=wb[:],
                in_=bass.AP(tensor=ext, offset=0,
                            ap=[[KCH * CO, P], [1, KCH * CO]]),
            )
            w_sb["w" + name] = wb

        def load_tab(nm, eng):
            t_ = consts.tile([P, T], BF16, tag=nm)
            eng.dma_start(out=t_[:], in_=tab_ext[nm][:, :])
            tabs[nm] = t_

        # Pool ring: tiny bias columns, then the four tables, bv, late xq1
        bias_cols = {}
        for nm in ("bq", "bk"):
            for ct in range(NCT):
                t_ = consts.tile([P, 1], F32, tag=f"{nm}{ct}")
                nc.gpsimd.dma_start(
                    out=t_[:], in_=bias_ext[nm][ct * P : (ct + 1) * P, :]
                )
                bias_cols[(nm, ct)] = t_
        load_tab("cosq", nc.gpsimd)
        load_tab("sinq", nc.gpsimd)
        load_tab("cosk", nc.gpsimd)
        load_tab("sink", nc.gpsimd)
        bvb_sb = consts.tile([P, CO], F32, tag="bvb")
        nc.gpsimd.dma_start(
            out=bvb_sb[:],
            in_=bass.AP(tensor=bias_ext["bv"], offset=0, ap=[[0, P], [1, CO]]),
        )
        # Act ring: ONLY early-completing pieces (wq/wk/wv + h0 middles) so
        # the in-order Act sequencer is free for the exps from ~20us on
        load_w("q", wq_ext, nc.scalar)
        load_x_part("q", xq_ext, 0, 3, 6, nc.scalar)
        load_w("k", wk_ext, nc.scalar)
        load_x_part("kv", xkv_ext, 0, 3, 6, nc.scalar)
        load_w("v", wv_ext, nc.scalar)
        # SP ring: h0 head/tail thirds, then all of h1 (kv before q --
        # k01 is woven before q01)
        load_x_part("q", xq_ext, 0, 0, 3, nc.sync)
        load_x_part("kv", xkv_ext, 0, 0, 3, nc.sync)
        load_x_part("q", xq_ext, 0, 6, 8, nc.sync)
        load_x_part("kv", xkv_ext, 0, 6, 8, nc.sync)
        load_x_part("kv", xkv_ext, 1, 0, 4, nc.sync)
        load_x_part("kv", xkv_ext, 1, 4, 8, nc.sync)
        load_x_part("q", xq_ext, 1, 0, 4, nc.sync)
        # late xq1 second half on the Pool ring (after the tables)
        load_x_part("q", xq_ext, 1, 4, 8, nc.gpsimd)
        # persistent rotated q/k and v tiles
        rot_sb = {}
        for name in ("q", "k"):
            for ct in range(NCT):
                for hf in range(XH):
                    rot_sb[(name, ct, hf)] = persist.tile(
                        [P, TH], BF16, tag=f"r{name}{ct}{hf}",
                        name=f"r{name}{ct}{hf}"
                    )
        v_sb = [
            persist.tile([P, HEADS_PER_CORE, HEAD_DIM + 1], BF16,
                         tag=f"v{st}", name=f"v{st}")
            for st in range(NST)
        ]

        stack_p = ExitStack()
        projp = stack_p.enter_context(
            tc.tile_pool(name="projp", bufs=2, space="PSUM")
        )

        # ------------- q/k projection + rotary (one t-half) -------------
        # proj psum is ONE [P, TH] tile (2 banks; the two TCH chunks are
        # independent accumulation groups in separate banks), so the
        # rotary runs as a single whole-tile op chain on the DVE.
        def proj_alloc(nm):
            return projp.tile([P, TH], F32, tag="pj", name=f"pj{nm}")

        def proj_mms(name, xsrc, ct, half, ps, ks):
            for k in ks:
                for i in range(2):
                    nc.tensor.matmul(
                        ps[:, i * TCH : (i + 1) * TCH],
                        w_sb["w" + name][:, k, ct * P : (ct + 1) * P],
                        x_sb[(xsrc, half)][:, k, i * TCH : (i + 1) * TCH],
                        start=(k == 0),
                        stop=(k == KCH - 1),
                    )

        def proj_rot(name, ct, half, ps):
            # qb = bf16(q + b) evacuates PSUM; shuffle(qb) = swap(q)+swap(b)
            # so the sin path needs no separate swapped-bias table.
            dst = rot_sb[(name, ct, half)]
            tsl = slice(half * TH, (half + 1) * TH)
            qb = projtmp.tile([P, TH], BF16, tag="qb",
                              name=f"qb{name}{ct}{half}")
            nc.vector.tensor_scalar_add(
                qb[:], ps[:], bias_cols[("b" + name, ct)][:]
            )
            qsw = projtmp.tile([P, TH], BF16, tag="qsw",
                               name=f"qsw{name}{ct}{half}")
            nc.vector.stream_shuffle(qsw[:], qb[:], _SWAP_MASK)
            t2 = projtmp.tile([P, TH], BF16, tag="rot2",
                              name=f"t2{name}{ct}{half}")
            nc.vector.tensor_mul(t2[:], qsw[:], tabs["sin" + name][:, tsl])
            nc.vector.tensor_mul(dst[:], qb[:], tabs["cos" + name][:, tsl])
            nc.vector.tensor_add(dst[:], dst[:], t2[:])

        def proj_group(name, xsrc, ct, half):
            ps = proj_alloc(f"{name}{ct}{half}")
            proj_mms(name, xsrc, ct, half, ps, range(KCH))
            proj_rot(name, ct, half, ps)

        wps = projp.tile([P, TCH], F32, tag="pj", name="warmps")
        for i in range(40):
            nc.tensor.matmul(
                wps[:], warm[:, 0:P], warm[:],
                start=True, stop=True,
            )

        # v projection ([128,256] fits a projp slot)
        def emit_v_proj(sts):
            for st in sts:
                vt = v_sb[st]
                psv = projp.tile([P, CO], F32, tag="pj", name=f"psv{st}")
                half, col = divmod(st * P, TH)
                for k in range(KCH):
                    nc.tensor.matmul(
                        psv[:],
                        x_sb[("kv", half)][:, k, col : col + P],
                        w_sb["wv"][:, k, :],
                        start=(k == 0),
                        stop=(k == KCH - 1),
                    )
                nc.vector.tensor_add(
                    vt[:, :, 0:HEAD_DIM],
                    psv[:].rearrange("p (h d) -> p h d", h=HEADS_PER_CORE),
                    bvb_sb[:].rearrange("p (h d) -> p h d", h=HEADS_PER_CORE),
                )
                nc.vector.memset(vt[:, :, HEAD_DIM : HEAD_DIM + 1], 1.0)

        # ---------------- attention ----------------
        ITERS = [(p_, t_) for p_ in range(NCT) for t_ in range(2)]

        def scores_exp(it, st):
            pair, th = ITERS[it]
            rk = rot_sb[("k", pair, st // 8)]
            rq = rot_sb[("q", pair, th)]
            so = (st % 8) * P
            pss = [
                scp.tile([P, TH], F32, tag="sc", name=f"sc{it}_{st}_{h}")
                for h in range(2)
            ]
            # one [k=64, m=128, n=512] matmul per (head, bank): 64x128
            # row-tiled mode; h0 on row-tile 0, h1 on row-tile 64.
            for tcc in range(2):
                psl = slice(tcc * TCH, (tcc + 1) * TCH)
                for h in range(2):
                    nc.tensor.matmul(
                        pss[h][:, psl],
                        rk[h * 64 : (h + 1) * 64, so : so + P],
                        rq[h * 64 : (h + 1) * 64, psl],
                        start=True, stop=True,
                        tile_position=(h * 64, 0),
                    )
            etiles = []
            for h in range(2):
                e = expp.tile([P, TH], BF16, tag="exp", name=f"e{it}_{st}_{h}")
                nc.scalar.activation(e[:], pss[h][:], ExpF, scale=SCALE)
                etiles.append(e)
            return etiles

        def av_mms(it, st, vps, etiles):
            pair, th = ITERS[it]
            for sub in range(2):
                h = pair * 2 + sub
                e = etiles[st][sub]
                for tcc in range(2):
                    psl = slice(tcc * TCH, (tcc + 1) * TCH)
                    nc.tensor.matmul(
                        vps[sub][:, psl],
                        v_sb[st][:, h, :],
                        e[:, psl],
                        start=(st == 0),
                        stop=(st == NST - 1),
                    )

        def epilogue_sub(it, vps, sub, tail=False):
            pair, th = ITERS[it]
            h = pair * 2 + sub
            vcp = outp.tile([HEAD_DIM + 1, TH], F32, tag="vcp",
                            name=f"vcp{it}_{sub}")
            nc.vector.tensor_copy(vcp[:], vps[sub][:])
            dn = smallp.tile([1, TH], F32, tag="dn",
                             name=f"dn{it}_{sub}")
            nc.sync.dma_start(
                out=dn[:], in_=vcp[HEAD_DIM : HEAD_DIM + 1, :]
            )
            if tail:
                # tail epilogue: the scores PSUM banks are free, so the
                # 1->64 broadcast is a PE outer product (ones64^T @ recb)
                # instead of two ~1us GpSimd broadcasts.
                nc.vector.reciprocal_approx_fast(out=dn[:], in_=dn[:])
                recb = smallp.tile([1, TH], BF16, tag="recb",
                                   name=f"recb{it}_{sub}")
                nc.vector.tensor_copy(recb[:], dn[:])
                bc = scp.tile([HEAD_DIM, TH], F32, tag="sc",
                              name=f"bc{it}_{sub}")
                for j in range(2):
                    nc.tensor.matmul(
                        bc[:, j * TCH : (j + 1) * TCH],
                        ones64[:],
                        recb[:, j * TCH : (j + 1) * TCH],
                        start=True, stop=True,
                    )
                nc.vector.tensor_mul(
                    vcp[0:HEAD_DIM, :], vcp[0:HEAD_DIM, :], bc[:]
                )
            else:
                nc.vector.reciprocal_approx_fast(out=dn[:], in_=dn[:])
                recb = smallp.tile([1, TH], BF16, tag="recb",
                                   name=f"recb{it}_{sub}")
                nc.vector.tensor_copy(recb[:], dn[:])
                # broadcast 1->64 partitions on GpSimd (keeps PE queue free)
                rcb = smallp.tile([HEAD_DIM, TH], BF16, tag="rcb",
                                  name=f"rcb{it}_{sub}")
                for j in range(2):
                    nc.gpsimd.partition_broadcast(
                        rcb[:, j * TCH : (j + 1) * TCH],
                        recb[:, j * TCH : (j + 1) * TCH],
                        channels=HEAD_DIM,
                    )
                nc.vector.tensor_mul(
                    vcp[0:HEAD_DIM, :], vcp[0:HEAD_DIM, :], rcb[:]
                )
            nc.sync.dma_start(
                out=out_ext[h * HEAD_DIM : (h + 1) * HEAD_DIM,
                            th * TH : (th + 1) * TH],
                in_=vcp[0:HEAD_DIM, :],
            )

        def epilogue(it, vps):
            for sub in range(2):
                epilogue_sub(it, vps, sub)

        # ---------------- window it0 with woven projections -------------
        # pair-0 th0 groups first (critical path to the first exp); all
        # remaining projections (v x16, k01/q01, pair-1) are then woven
        # two-chunks-per-gap into the ACT-paced idle between score tiles,
        # ordered by when their DMA inputs land, so the in-order PE queue
        # never blocks the exp pipeline and the DVE reaches every rotary
        # early.
        proj_group("q", "q", 0, 0)
        proj_group("k", "kv", 0, 0)

        def pgroup_chunks(nm, src, ct, hf):
            # the psum tile is allocated by chunk1 at weave-execution time
            # so projp slot rotation matches the PE emission order
            state = {}

            def chunk1():
                state["ps"] = proj_alloc(f"{nm}{ct}{hf}")
                proj_mms(nm, src, ct, hf, state["ps"], range(0, 4))

            def chunk2():
                proj_mms(nm, src, ct, hf, state["ps"], range(4, KCH))
                proj_rot(nm, ct, hf, state["ps"])

            return [chunk1, chunk2]

        weave = []
        for st_ in range(0, 8):
            weave.append(lambda s=st_: emit_v_proj([s]))
        weave += pgroup_chunks("k", "kv", 0, 1)
        weave += pgroup_chunks("q", "q", 0, 1)
        for st_ in range(8, NST):
            weave.append(lambda s=st_: emit_v_proj([s]))
        weave += pgroup_chunks("q", "q", 1, 0)
        weave += pgroup_chunks("k", "kv", 1, 0)
        weave += pgroup_chunks("q", "q", 1, 1)
        weave += pgroup_chunks("k", "kv", 1, 1)

        et = {0: []}
        wq_i = 0
        for st in range(NST):
            et[0].append(scores_exp(0, st))
            if st >= 1:
                for _ in range(2):
                    if wq_i < len(weave):
                        weave[wq_i]()
                        wq_i += 1
        while wq_i < len(weave):
            weave[wq_i]()
            wq_i += 1
        stack_p.close()

        psva = stack_all.enter_context(
            tc.tile_pool(name="psva", bufs=2, space="PSUM")
        )

        def new_vps(it):
            return [
                psva.tile([HEAD_DIM + 1, TH], F32, tag="va",
                          name=f"vacc{it}_{s}")
                for s in range(2)
            ]

        # windows it1..it3.  AV(it) runs one window after its exps, woven
        # two-st-per-gap into the late gaps (its accumulators only become
        # available once the projection pool has fully drained / the
        # previous epilogue has read the accumulator banks).
        vps = {}
        vps[0] = new_vps(0)
        et[1] = []
        for st in range(NST):
            et[1].append(scores_exp(1, st))
            if st >= 8:
                av_mms(0, 2 * (st - 8), vps[0], et[0])
                av_mms(0, 2 * (st - 8) + 1, vps[0], et[0])
        epilogue(0, vps[0])
        vps[1] = new_vps(1)

        et[2] = []
        for st in range(NST):
            et[2].append(scores_exp(2, st))
            if 1 <= st <= 8:
                av_mms(1, 2 * (st - 1), vps[1], et[1])
                av_mms(1, 2 * (st - 1) + 1, vps[1], et[1])
                if st == 8:
                    epilogue(1, vps[1])
                    vps[2] = new_vps(2)
            elif st >= 9:
                av_mms(2, st - 9, vps[2], et[2])
        av_mms(2, 7, vps[2], et[2])

        et[3] = []
        for st in range(NST):
            et[3].append(scores_exp(3, st))
            if st <= 3:
                av_mms(2, 8 + 2 * st, vps[2], et[2])
                av_mms(2, 9 + 2 * st, vps[2], et[2])
            elif st == 4:
                epilogue(2, vps[2])
                vps[3] = new_vps(3)
            elif st >= 8:
                av_mms(3, 2 * (st - 8), vps[3], et[3])
                av_mms(3, 2 * (st - 8) + 1, vps[3], et[3])
        # tail: just the two epilogue chains (PE-broadcast variant),
        # nothing queued behind them on the PE.
        for sub in range(2):
            epilogue_sub(3, vps[3], sub, tail=True)

        stack_all.close()
    nc.finalize()
    return nc


def make_in_maps(x_q, x_kv, q_positions, kv_positions, Wq, bq, Wk, bk, Wv, bv):
    x_q = np.asarray(x_q, np.float32)
    x_kv = np.asarray(x_kv, np.float32)
    q_positions = np.asarray(q_positions, np.int32)
    kv_positions = np.asarray(kv_positions, np.int32)
    Wq, Wk, Wv = (np.asarray(w, np.float32) for w in (Wq, Wk, Wv))
    bq, bk, bv = (np.asarray(b, np.float32) for b in (bq, bk, bv))

    xqT = [np.ascontiguousarray(x_q[b_].T).astype(NPBF16) for b_ in range(B)]
    xkvT = [np.ascontiguousarray(x_kv[b_].T).astype(NPBF16) for b_ in range(B)]
    tabs = []
    for b_ in range(B):
        cq, sq = _rot_tables(q_positions[b_])
        ck, sk = _rot_tables(kv_positions[b_])
        tabs.append((cq, sq, ck, sk))

    in_maps = []
    for core in range(N_CORES):
        b_, hg = divmod(core, N_CORES // B)
        hsl = slice(hg * CO, (hg + 1) * CO)
        cq, sq, ck, sk = tabs[b_]
        def prepack(wT):  # [C, n] -> [P, KCH, n] partition-major
            n = wT.shape[1]
            return np.ascontiguousarray(
                wT.reshape(KCH, P, n).transpose(1, 0, 2)
            )

        m = {
            "wqT": prepack(Wq[hsl].T.astype(NPBF16)),
            "wkT": prepack(Wk[hsl].T.astype(NPBF16)),
            "wvT": prepack(Wv[hsl].T.astype(NPBF16)),
            "bq": np.ascontiguousarray(bq[hsl][:, None]),
            "bk": np.ascontiguousarray(bk[hsl][:, None]),
            "bv": np.ascontiguousarray(bv[hsl][:, None]),
            "cosq": cq, "sinq": sq, "cosk": ck, "sink": sk,
        }
        for h in range(XH):
            m[f"xqT{h}"] = prepack(xqT[b_][:, h * TH : (h + 1) * TH])
            m[f"xkvT{h}"] = prepack(xkvT[b_][:, h * TH : (h + 1) * TH])
        in_maps.append(m)
    return in_maps


_CACHED = {}


def kernel(x_q, x_kv, q_positions, kv_positions, Wq, bq, Wk, bk, Wv, bv):
    in_maps = make_in_maps(
        x_q, x_kv, q_positions, kv_positions, Wq, bq, Wk, bk, Wv, bv
    )
    if "nc" not in _CACHED:
        _CACHED["nc"] = build_bass()
    nc = _CACHED["nc"]

    res = run_bass_kernel_spmd(nc, in_maps, core_ids=list(range(N_CORES)))
    out = np.empty((B, T, C), np.float32)
    for core in range(N_CORES):
        b_, hg = divmod(core, N_CORES // B)
        out[b_, :, hg * CO : (hg + 1) * CO] = res.results[core]["out"].T
    return out
